# revision 38
# baseline (speedup 1.0000x reference)
"""Bipartite GNN message-passing kernel for 8 TRN2 NeuronCores (v2).

Strategy (per core, 98 blocks of 128 destination nodes each):
  - Host: sort edges by destination (cons for pass 1, vars for pass 2),
    group into 128-node blocks, pad to uniform T tiles of 128 edges.
    Pack per-edge raw features in a 2-group block-diagonal layout
    (24/20 partition rows x 384-wide chunks, ones-row for baked biases).
  - Device pass 1 (cons-sorted): one feature DMA per block; per chunk of
    768 edges: one embed matmul -> relu -> join L1 -> relu -> join L2 ->
    relu -> 6 PE transposes to edge-major -> one-hot (is_equal) segment
    scatter matmuls accumulated in PSUM per block; rep layer -> oc2.
  - AllGather oc2 row-major slices; pass 2 (vars-sorted): ONE batched
    indirect DMA gather per block (idx [128,T], flat out AP), PE
    transposes to feature-major, streamed raw B-side features, same join
    + scatter + rep + output MLP.
  - Elementwise work is spread across Act/DVE/Pool engines; biases are
    baked into matmuls via ones-rows or fused into tensor_scalar ops.
"""

import sys

sys.path.insert(0, "/opt/trn_rl_repo")

import inspect
import textwrap

import numpy as np
import ml_dtypes

import concourse.bass as bass
import concourse.tile as tile
from concourse import bacc, mybir

# dma_gather with the 256B *payload* restriction relaxed (the 256B table
# *stride* restriction is real HW ucode behavior and is kept).  Verified on
# HW: elem_size=32 bf16 payload with elem_step=128 (256B stride) gathers
# correctly when the int16 indices are replicated across the 8 x 16-partition
# groups (each GPSIMD core reads its own 16 partitions).
_src = textwrap.dedent(inspect.getsource(bass.BassGpSimd.dma_gather))
_src = _src.replace(
    "assert (\n"
    "        elem_size_bytes > 0 and elem_size_bytes % 256 == 0\n"
    "    )  # transpose restriction",
    "assert elem_size_bytes > 0")
assert "% 256 == 0" not in _src
_ns = dict(vars(bass))
exec(compile(_src, "<patched_dma_gather>", "exec"), _ns)
bass.BassGpSimd.dma_gather_small = _ns["dma_gather"]

RNG = 32768  # int16 index range per gather run (cons >> 15)

# The stock cost model says SWDGE descriptor generation costs 0.34ns/desc;
# measured on HW it is ~7.5ns/desc (the Tile scheduler uses this model to
# order instructions -- with the stock value it schedules gathers lazily).
from concourse import hw_specs as _hw_specs

_hw_specs.TRN2Spec.SWDGE_NS_PER_DESCRIPTOR = 7.5

BF16 = ml_dtypes.bfloat16
P = 128
D = 32
CT = 6          # tiles per chunk (2 groups x 3)
W = 384         # free width per chunk (3 tiles)

F32 = mybir.dt.float32
BF = mybir.dt.bfloat16
I32 = mybir.dt.int32
I16 = mybir.dt.int16
Relu = mybir.ActivationFunctionType.Relu
Copy = mybir.ActivationFunctionType.Copy
Add = mybir.AluOpType.add
Max = mybir.AluOpType.max
IsEq = mybir.AluOpType.is_equal


# ----------------------------------------------------------------------------
# host-side preprocessing
# ----------------------------------------------------------------------------

def _prep(sort_key, cores, bpc):
    """Sort edges by sort_key, assign per-block slots, T mult of CT."""
    E = sort_key.shape[0]
    nblk = cores * bpc
    order = np.argsort(sort_key, kind="stable")
    sk = sort_key[order].astype(np.int64)
    blk = sk // P
    cnt = np.bincount(blk, minlength=nblk).astype(np.int64)
    T = int(np.ceil(cnt.max() / P))
    T = (T + CT - 1) // CT * CT
    starts = np.zeros(nblk + 1, np.int64)
    np.cumsum(cnt, out=starts[1:])
    rank = np.arange(E, dtype=np.int64) - starts[blk]
    return order, sk, blk, rank, nblk, T


def _slot_cols(rank, T):
    """tile t = 6c+3g+j -> (t, g, column c*384 + j*128 + p)."""
    t = rank // P
    p = rank % P
    c = t // CT
    r = t % CT
    g = r // 3
    j = r % 3
    col = c * W + j * P + p
    return t, p, g, col


def _pack_rid(blk, sk, rank, nblk, T, cores, bpc):
    """[cores, 128, bpc*T] bf16: node offset within block, 999 for pads."""
    t, p, _, _ = _slot_cols(rank, T)
    arr = np.full((nblk, P, T), 999.0, np.float32)
    arr[blk, p, t] = (sk - blk * P).astype(np.float32)
    arr = arr.reshape(cores, bpc, P, T).transpose(0, 2, 1, 3)
    return np.ascontiguousarray(arr.reshape(cores, P, bpc * T).astype(BF16))


def _prep2_runs(ec, ev, cores, bpc):
    """Vars-sorted edges, run-partitioned by cons>>15 inside each block.

    Each block's edges are grouped into 4 runs by cons index range (so the
    per-run dma_gather int16 indices stay < 32768), runs padded to whole
    128-edge tiles with tile counts uniform across cores (SPMD program).
    """
    E = ev.shape[0]
    nblk = cores * bpc
    o = np.argsort(ev, kind="stable")
    sk = ev[o].astype(np.int64)
    blk = sk // P
    cons = ec[o].astype(np.int64)
    s = cons // RNG
    NR = 4
    order2 = np.lexsort((np.arange(E), s, blk))
    o, sk, blk, cons, s = (o[order2], sk[order2], blk[order2],
                           cons[order2], s[order2])
    len_bs = np.zeros((nblk, NR), np.int64)
    np.add.at(len_bs, (blk, s), 1)
    nt = np.ceil(len_bs.reshape(cores, bpc, NR).max(axis=0) / P)
    nt = nt.astype(np.int64)                      # [bpc, NR]
    off = np.zeros((bpc, NR), np.int64)
    off[:, 1:] = np.cumsum(nt, axis=1)[:, :-1]
    Tu = nt.sum(axis=1)
    T = int(Tu.max())
    T = (T + 2 * CT - 1) // (2 * CT) * (2 * CT)  # even chunk count
    grp = blk * NR + s
    starts = np.zeros(nblk * NR + 1, np.int64)
    np.cumsum(np.bincount(grp, minlength=nblk * NR), out=starts[1:])
    pos = np.arange(E, dtype=np.int64) - starts[grp]
    rank = off[blk % bpc, s] * P + pos
    return o, sk, blk, rank, cons, s, nt, T


def _pack_idx16(blk, rank, cons, s, nblk, T, cores, bpc):
    """[cores, bpc, 128, T*8] int16 gather indices, replicated across the
    8x16-partition groups (each GPSIMD core reads its own 16 partitions)."""
    C = T * 8
    arr = np.zeros((nblk, 16, C), np.int16)
    arr[blk, rank % 16, rank // 16] = (cons - s * RNG).astype(np.int16)
    arr = arr.reshape(cores, bpc, 16, C)
    return np.ascontiguousarray(np.tile(arr, (1, 1, 8, 1)))


def _pack_groupfeats(blk, rank, T, nblk, cores, bpc, row_blocks):
    """Pack per-edge features into [cores, bpc, GR, (T//CT)*W] bf16.

    row_blocks: list of (feat_array [E_sorted, F], row_offset_in_group).
    Group g occupies partition rows [g*gstride, (g+1)*gstride); the last
    row of each group is set to 1.0 (bias ones-row).
    """
    gstride = sum(f.shape[1] for f, _ in row_blocks) + 1
    GR = 2 * gstride
    t, p, g, col = _slot_cols(rank, T)
    nch = T // CT
    arr = np.zeros((nblk, GR, nch * W), np.float32)
    for feat, roff in row_blocks:
        F = feat.shape[1]
        for f in range(F):
            arr[blk, g * gstride + roff + f, col] = feat[:, f]
    arr[:, gstride - 1, :] = 1.0
    arr[:, 2 * gstride - 1, :] = 1.0
    out = arr.reshape(cores, bpc, GR, nch * W).astype(BF16)
    return np.ascontiguousarray(out), GR


def preprocess(edge_indices, cons_features, vars_features, cores, bpc):
    ec = np.asarray(edge_indices[0]).astype(np.int64)
    ev = np.asarray(edge_indices[1]).astype(np.int64)

    # pass 1: sorted by cons
    o1, sk1, bk1, rk1, nblk, T1 = _prep(ec, cores, bpc)
    f1, GR1 = _pack_groupfeats(
        bk1, rk1, T1, nblk, cores, bpc,
        [(cons_features[ec[o1]], 0), (vars_features[ev[o1]], 2)])
    rid1 = _pack_rid(bk1, sk1, rk1, nblk, T1, cores, bpc)

    # pass 2: sorted by vars, run-partitioned by cons range for dma_gather
    nblk = cores * bpc
    o2, sk2, bk2, rk2, cons2, s2, nt2, T2 = _prep2_runs(ec, ev, cores, bpc)
    f2, GR2 = _pack_groupfeats(
        bk2, rk2, T2, nblk, cores, bpc, [(vars_features[ev[o2]], 0)])
    rid2 = _pack_rid(bk2, sk2, rk2, nblk, T2, cores, bpc)
    ia2 = _pack_idx16(bk2, rk2, cons2, s2, nblk, T2, cores, bpc)
    gather_plan = [[int(v) for v in nt2[lb]] for lb in range(bpc)]

    return (f1, GR1, rid1, T1), (f2, GR2, rid2, ia2, T2, gather_plan)


def _bdj1id(w):
    """L1 weight for pass 2: A-half = identity (gathered A2 passes through),
    B-half = Wj1B, 2-group block-diagonal."""
    m = np.zeros((P, 64), np.float32)
    for g in range(2):
        m[g * D:(g + 1) * D, g * D:(g + 1) * D] = np.eye(D)
        m[64 + g * D:64 + (g + 1) * D, g * D:(g + 1) * D] = w["Wj1"][D:2 * D]
    return m.astype(BF16)


def _host_weights(inputs, CF, VF):
    """Block-diagonal / packed weight arrays (host-side relayout only)."""
    w = {k: np.asarray(inputs[k], np.float32) for k in
         ["Wc", "bc", "Wv", "bv", "Wj1", "bj1", "Wj2", "bj2",
          "Wcr", "bcr", "Wvr", "bvr", "Wo1", "bo1", "Wo2", "bo2",
          "Wo3", "bo3"]}
    g1 = CF + VF + 1          # 12
    bdW0 = np.zeros((2 * g1, P), np.float32)
    for g in range(2):
        a0, b0 = g * D, 64 + g * D
        bdW0[g * g1:g * g1 + CF, a0:a0 + D] = w["Wc"]
        bdW0[g * g1 + CF:g * g1 + CF + VF, b0:b0 + D] = w["Wv"]
        bdW0[g * g1 + g1 - 1, a0:a0 + D] = w["bc"]
        bdW0[g * g1 + g1 - 1, b0:b0 + D] = w["bv"]
    g2 = VF + 1               # 10
    bdW0v = np.zeros((2 * g2, 64), np.float32)
    for g in range(2):
        bdW0v[g * g2:g * g2 + VF, g * D:(g + 1) * D] = w["Wv"]
        bdW0v[g * g2 + VF, g * D:(g + 1) * D] = w["bv"]
    bdJ1 = np.zeros((P, 64), np.float32)
    for g in range(2):
        bdJ1[g * D:(g + 1) * D, g * D:(g + 1) * D] = w["Wj1"][0:D]       # A
        bdJ1[64 + g * D:64 + (g + 1) * D, g * D:(g + 1) * D] = w["Wj1"][D:2 * D]
    bdJ2 = np.zeros((P, P), np.float32)
    for g in range(4):
        bdJ2[g * D:(g + 1) * D, g * D:(g + 1) * D] = w["Wj2"]

    com = {
        "bdW0": bdW0.astype(BF16), "bdW0v": bdW0v.astype(BF16),
        "bdJ1": bdJ1.astype(BF16), "bdJ2": bdJ2.astype(BF16),
        "WcrT": w["Wcr"][0:D].astype(BF16), "WcrB": w["Wcr"][D:2 * D].astype(BF16),
        "WvrT": w["Wvr"][0:D].astype(BF16), "WvrB": w["Wvr"][D:2 * D].astype(BF16),
        "Wo1": w["Wo1"].astype(BF16), "Wo2": w["Wo2"].astype(BF16),
        "Wo3": w["Wo3"].astype(BF16),
        "WjA": w["Wj1"][0:D].astype(BF16),
        "bdJ1ID": _bdj1id(w),
        "bj1v": w["bj1"].reshape(D, 1),
        "bj1_4": np.tile(w["bj1"], 4).reshape(P, 1),
        "bj2_4": np.tile(w["bj2"], 4).reshape(P, 1),
        "bcr": w["bcr"].reshape(D, 1), "bvr": w["bvr"].reshape(D, 1),
        "bo1": w["bo1"].reshape(D, 1), "bo2": w["bo2"].reshape(D, 1),
        "bo3": w["bo3"].reshape(1, 1),
        "ident": np.eye(P, dtype=BF16),
        "iota": np.arange(P, dtype=BF16).reshape(1, P),
    }
    return com


# ----------------------------------------------------------------------------
# device program
# ----------------------------------------------------------------------------

def _bcast_row(ap, parts=P):
    return bass.AP(tensor=ap.tensor, offset=ap.offset, ap=[[0, parts]] + ap.ap[1:])


def build_program(cores, bpc, T1, T2, CF, VF, gather_plan=None, debug=False):
    npad = cores * bpc * P
    ns = bpc * P
    GR1 = 2 * (CF + VF + 1)
    GR2 = 2 * (VF + 1)
    nch1, nch2 = T1 // CT, T2 // CT
    nc = bacc.Bacc(None, num_devices=cores)

    def inp(name, shape, dt):
        return nc.dram_tensor(name, shape, dt, kind="ExternalInput")

    t = {}
    t["consT"] = inp("consT", [CF, ns], BF)
    t["varsT"] = inp("varsT", [VF, ns], BF)
    t["f1"] = inp("f1", [bpc, GR1, nch1 * W], BF)
    t["f2"] = inp("f2", [bpc, GR2, nch2 * W], BF)
    t["rid1"] = inp("rid1", [P, bpc * T1], BF)
    t["rid2"] = inp("rid2", [P, bpc * T2], BF)
    t["ia2"] = inp("ia2", [bpc, P, T2 * 8], I16)
    for nm, shp in [("bdW0", [GR1, P]), ("bdW0v", [GR2, 64]),
                    ("bdJ1", [P, 64]), ("bdJ2", [P, P]),
                    ("WcrT", [D, D]), ("WcrB", [D, D]),
                    ("WjA", [D, D]), ("bdJ1ID", [P, 64]),
                    ("WvrT", [D, D]), ("WvrB", [D, D]),
                    ("Wo1", [D, D]), ("Wo2", [D, D]), ("Wo3", [D, 1]),
                    ("ident", [P, P]), ("iota", [1, P])]:
        t[nm] = inp(nm, shp, BF)
    for nm, shp in [("bj1_4", [P, 1]), ("bj2_4", [P, 1]),
                    ("bj1v", [D, 1]),
                    ("bcr", [D, 1]), ("bvr", [D, 1]),
                    ("bo1", [D, 1]), ("bo2", [D, 1]), ("bo3", [1, 1])]:
        t[nm] = inp(nm, shp, F32)

    out_t = nc.dram_tensor("out", [1, ns], F32, kind="ExternalOutput")
    oc2_row_slice = nc.dram_tensor("oc2_row_slice", [ns, D], BF, kind="Internal")
    oc2_row_full = nc.dram_tensor(
        "oc2_row_full", [npad, D], BF, kind="Internal",
        addr_space="Shared" if cores > 4 else "Local")
    # 256B-stride gather tables (one per 32768-row cons range)
    tabs = [nc.dram_tensor(f"oc2tab{s}", [RNG, P], BF, kind="Internal")
            for s in range(4)]

    dbg = {}
    if debug:
        dbg["dbg_oc2"] = (nc.dram_tensor("dbg_oc2", [npad, D], BF,
                                         kind="ExternalOutput"), oc2_row_full)

    if gather_plan is None:
        gather_plan = [[T2, 0, 0, 0]] * bpc
    with tile.TileContext(nc) as tc:
        _emit(tc, t, out_t, oc2_row_slice, oc2_row_full, tabs,
              cores, bpc, T1, T2, CF, VF, gather_plan)
        for nm, (dst, src_t) in dbg.items():
            nc.sync.dma_start(out=dst[:], in_=src_t[:])
    nc.compile()
    return nc


def _nested_ap(base, extra_dims_after_partition):
    """Insert broadcast/repeat dims after the partition dim of an SBUF AP."""
    return bass.AP(tensor=base.tensor, offset=base.offset,
                   ap=[base.ap[0]] + extra_dims_after_partition)


def _emit(tc, t, out_t, oc2_row_slice, oc2_row_full, tabs,
          cores, bpc, T1, T2, CF, VF, gather_plan):
    nc = tc.nc
    ns = bpc * P
    GR1 = 2 * (CF + VF + 1)
    GR2 = 2 * (VF + 1)
    nch1, nch2 = T1 // CT, T2 // CT
    from contextlib import ExitStack
    es = ExitStack()
    singles = es.enter_context(tc.tile_pool(name="singles", bufs=1))

    # ---- persistent SBUF ----
    ident = singles.tile([P, P], BF)
    nc.sync.dma_start(out=ident[:], in_=t["ident"][:])
    iota = singles.tile([P, P], BF)
    nc.sync.dma_start(out=iota[:], in_=_bcast_row(t["iota"][:]))

    def load(nm, shape, dt=BF):
        w = singles.tile(shape, dt, tag=nm)
        nc.sync.dma_start(out=w[:], in_=t[nm][:])
        return w

    bdW0 = load("bdW0", [GR1, P])
    bdW0v = load("bdW0v", [GR2, 64])
    bdJ1 = load("bdJ1", [P, 64])
    bdJ2 = load("bdJ2", [P, P])
    WcrT = load("WcrT", [D, D])
    WjA = load("WjA", [D, D])
    bdJ1ID = load("bdJ1ID", [P, 64])
    bj1v = load("bj1v", [D, 1], F32)
    WcrB = load("WcrB", [D, D])
    WvrT = load("WvrT", [D, D])
    WvrB = load("WvrB", [D, D])
    Wo1 = load("Wo1", [D, D])
    Wo2 = load("Wo2", [D, D])
    Wo3 = load("Wo3", [D, 1])
    bj1_4 = load("bj1_4", [P, 1], F32)
    bj2_4 = load("bj2_4", [P, 1], F32)
    bcr = load("bcr", [D, 1], F32)
    bvr = load("bvr", [D, 1], F32)
    bo1 = load("bo1", [D, 1], F32)
    bo2 = load("bo2", [D, 1], F32)
    bo3 = load("bo3", [1, 1], F32)
    rid1 = load("rid1", [P, bpc * T1])
    rid2 = load("rid2", [P, bpc * T2])

    cT = singles.tile([D, ns], BF)       # c = relu(cons @ Wc + bc), feat-major
    vT = singles.tile([D, ns], BF)
    a2T = singles.tile([D, ns], BF)
    out_sb = singles.tile([1, ns], F32)

    # ---- stage A: node tables ----
    WcT = singles.tile([CF, D], BF)
    nc.sync.dma_start(out=WcT[:], in_=t["bdW0"][0:CF, 0:D])
    WvT = singles.tile([VF, D], BF)
    nc.sync.dma_start(out=WvT[:], in_=t["bdW0"][CF:CF + VF, 64:64 + D])
    bc_sb = singles.tile([1, D], BF)
    nc.sync.dma_start(out=bc_sb[:], in_=t["bdW0"][CF + VF:CF + VF + 1, 0:D])
    bv_sb = singles.tile([1, D], BF)
    nc.sync.dma_start(out=bv_sb[:], in_=t["bdW0"][CF + VF:CF + VF + 1, 64:64 + D])

    ones1 = singles.tile([1, 512], BF, tag="ones1")
    nc.vector.memset(ones1[:], 1.0)
    with tc.tile_pool(name="bld", bufs=1) as bld, \
         tc.tile_pool(name="bps", bufs=4, space="PSUM") as bps:
        consT = bld.tile([CF, ns], BF, tag="consT")
        nc.sync.dma_start(out=consT[:], in_=t["consT"][:])
        varsT = bld.tile([VF, ns], BF, tag="varsT")
        nc.sync.dma_start(out=varsT[:], in_=t["varsT"][:])
        for st0 in range(0, ns, 512):
            wdt = min(512, ns - st0)
            for (Wsb, src, bias, dst) in [(WcT, consT, bc_sb, cT),
                                          (WvT, varsT, bv_sb, vT)]:
                ps = bps.tile([D, 512], F32, tag="ps")
                nc.tensor.matmul(out=ps[:, :wdt], lhsT=Wsb[:],
                                 rhs=src[:, st0:st0 + wdt], start=True, stop=False)
                nc.tensor.matmul(out=ps[:, :wdt], lhsT=bias[:],
                                 rhs=ones1[:, :wdt], start=False, stop=True)
                nc.scalar.activation(dst[:, st0:st0 + wdt], ps[:, :wdt], Relu)

    # ---- shared chunk emitter (chunks processed in pairs) ----
    def do_chunks(T, x_cb, rid, rid_off, out_ps, sb, psp):
        nchk = T // CT
        assert nchk % 2 == 0
        for pr in range(nchk // 2):
            c0ch = 2 * pr
            xs = [x_cb(c0ch), x_cb(c0ch + 1)]
            # join L1 for both chunks into one [128, W] psum
            ps_h = psp.tile([P, W], F32, tag="psh")
            for q2 in range(2):
                nc.tensor.matmul(out=ps_h[q2 * 64:(q2 + 1) * 64, :],
                                 lhsT=bdJ1[:], rhs=xs[q2][:],
                                 start=True, stop=True,
                                 tile_position=(0, q2 * 64),
                                 skip_group_check=True)
            h = sb.tile([P, W], BF, tag="h")
            nc.vector.tensor_scalar(out=h[:], in0=ps_h[:], scalar1=bj1_4[:],
                                    scalar2=0.0, op0=Add, op1=Max)
            # join L2 (4 block-diagonal groups)
            ps_j = psp.tile([P, W], F32, tag="psj")
            nc.tensor.matmul(out=ps_j[:], lhsT=bdJ2[:], rhs=h[:],
                             start=True, stop=True)
            j = sb.tile([P, W], BF, tag="j")
            nc.scalar.activation(j[:], ps_j[:], Relu, bias=bj2_4[:])
            # transpose to edge-major: col-slice jj -> [128, 128] holding
            # 4 row-groups q (chunk c0ch+q//2, group q%2)
            ps_e = psp.tile([P, W], BF, tag="pse")
            for jj in range(3):
                nc.tensor.matmul(
                    out=ps_e[:, jj * P:(jj + 1) * P],
                    lhsT=j[:, jj * P:(jj + 1) * P],
                    rhs=ident[:], is_transpose=True, skip_group_check=True)
            jem = sb.tile([P, W], BF, tag="jem")
            nc.scalar.activation(jem[:], ps_e[:], Copy)
            # one-hot scatter per chunk
            for dc in range(2):
                c = c0ch + dc
                cb = rid_off + c * CT
                S = sb.tile([P, CT * P], BF, tag="S")
                nc.vector.tensor_tensor(
                    out=S[:],
                    in0=_nested_ap(rid[:, cb:cb + CT], [[1, CT], [0, P]]),
                    in1=_nested_ap(iota[:], [[0, CT], [1, P]]),
                    op=IsEq)
                for g in range(2):
                    for jj in range(3):
                        u = 3 * g + jj
                        q = 2 * dc + g
                        nc.tensor.matmul(
                            out=out_ps[:],
                            lhsT=jem[:, jj * P + q * D:jj * P + (q + 1) * D],
                            rhs=S[:, u * P:(u + 1) * P],
                            start=(c == 0 and u == 0),
                            stop=(c == nchk - 1 and u == CT - 1),
                            skip_group_check=True)

    # ---- pass 1 ----
    with tc.tile_pool(name="p1f", bufs=3) as fpool, \
         tc.tile_pool(name="p1s", bufs=3) as sb, \
         tc.tile_pool(name="p1x", bufs=2, space="PSUM") as pspx, \
         tc.tile_pool(name="p1p", bufs=1, space="PSUM") as psp, \
         tc.tile_pool(name="p1o", bufs=2, space="PSUM") as opsp:
        for b in range(bpc):
            f_sb = fpool.tile([GR1, nch1 * W], BF, tag="f1")
            nc.sync.dma_start(out=f_sb[:], in_=t["f1"][b])
            out_ps = opsp.tile([D, P], F32, tag="ob")

            def x1(c, f_sb=f_sb):
                ps_x = pspx.tile([P, W], F32, tag="psx")
                nc.tensor.matmul(out=ps_x[:], lhsT=bdW0[:],
                                 rhs=f_sb[:, c * W:(c + 1) * W],
                                 start=True, stop=True)
                x = sb.tile([P, W], BF, tag="x")
                nc.scalar.activation(x[:], ps_x[:], Relu)
                return x

            do_chunks(T1, x1, rid1, b * T1, out_ps, sb, psp)
            # rep layer 1 -> oc2T
            oc_sb = sb.tile([D, P], BF, tag="oc")
            nc.scalar.activation(oc_sb[:], out_ps[:], Copy)
            ps_r = psp.tile([D, P], F32, tag="psr")
            nc.tensor.matmul(out=ps_r[:], lhsT=WcrT[:], rhs=oc_sb[:],
                             start=True, stop=False)
            nc.tensor.matmul(out=ps_r[:], lhsT=WcrB[:],
                             rhs=cT[:, b * P:(b + 1) * P],
                             start=False, stop=True)
            oc2_blk = sb.tile([D, P], BF, tag="oc2blk")
            nc.scalar.activation(oc2_blk[:], ps_r[:], Relu, bias=bcr[:])
            # A2 = Wj1A^T oc2 + bj1 (the pass-2 join L1 A-contribution):
            # exchanging A2 instead of oc2 lets gathered rows accumulate
            # straight into the L1 psum on the consumer side.
            ps_a2 = psp.tile([D, P], F32, tag="psr")
            nc.tensor.matmul(out=ps_a2[:], lhsT=WjA[:],
                             rhs=oc2_blk[:],
                             start=True, stop=True)
            nc.vector.tensor_scalar_add(a2T[:, b * P:(b + 1) * P],
                                        ps_a2[:], bj1v[:])

    # ---- exchange ----
    with tc.tile_pool(name="xch", bufs=1) as xp:
        stg = xp.tile([P, bpc, D], BF)
        nc.sync.dma_start(out=stg[:], in_=a2T[:], transpose=True)
        nc.sync.dma_start(
            out=oc2_row_slice[:].rearrange("(j p) d -> p j d", p=P),
            in_=stg[:])
    import os as _os
    if cores > 1 and not _os.environ.get("K_SKIP_COLLECTIVE"):
        nc.gpsimd.collective_compute(
            "AllGather", mybir.AluOpType.bypass,
            replica_groups=[list(range(cores))],
            ins=[oc2_row_slice[:]], outs=[oc2_row_full[:]])
    else:
        nc.sync.dma_start(out=oc2_row_full[0:bpc * P, :], in_=oc2_row_slice[:])
    # expand the compact [npad, D] table into 4 tables with 256B row stride
    import os as _os
    npad = cores * bpc * P
    if not _os.environ.get("K_SKIP_EXPAND"):
        for s in range(4):
            rows = min(RNG, npad - s * RNG)
            if rows <= 0:
                continue
            eng = nc.sync if s % 2 == 0 else nc.scalar
            eng.dma_start(out=tabs[s][0:rows, 0:D],
                          in_=oc2_row_full[s * RNG:s * RNG + rows, :])

    # ---- pass 2 ----
    with tc.tile_pool(name="p2f", bufs=3) as fpool, \
         tc.tile_pool(name="p2i", bufs=6) as ipool, \
         tc.tile_pool(name="p2g", bufs=6) as gpool, \
         tc.tile_pool(name="p2s", bufs=3) as sb, \
         tc.tile_pool(name="p2p", bufs=1, space="PSUM") as psp, \
         tc.tile_pool(name="p2o", bufs=2, space="PSUM") as opsp:
        for b in range(bpc):
            plan = gather_plan[b]
            tg = sum(plan)
            # per-block effective tile count (global T2 is the worst case)
            teff = (tg + 2 * CT - 1) // (2 * CT) * (2 * CT)
            nchb = teff // CT
            f_sb = fpool.tile([GR2, nch2 * W], BF, tag="f2")
            nc.sync.dma_start(out=f_sb[:, :nchb * W],
                              in_=t["f2"][b, :, 0:nchb * W])
            with tc.high_priority():
                it = ipool.tile([P, T2 * 8], I16, tag="it")
                nc.scalar.dma_start(out=it[:], in_=t["ia2"][b])
                g_sb = gpool.tile([P, T2 * D], BF, tag="gth")
                if tg < teff:
                    nc.vector.memset(g_sb[:, tg * D:teff * D], 0)
                offt = 0
                for s in range(4):
                    nts_all = plan[s]
                    if nts_all == 0:
                        continue
                    ncall = (nts_all + 7) // 8  # HW limit: ~1024 idx/call
                    step = (nts_all + ncall - 1) // ncall
                    left = nts_all
                    while left > 0:
                        nts = min(left, step)
                        sl = g_sb[:, offt * D:(offt + nts) * D]
                        g3 = bass.AP(tensor=sl.tensor, offset=sl.offset,
                                     ap=[sl.ap[0], [D, nts], [1, D]])
                        nc.gpsimd.dma_gather_small(
                            out_ap=g3,
                            in_ap=tabs[s][:, 0:D],
                            idxs_ap=it[:, offt * 8:(offt + nts) * 8],
                            num_idxs=nts * P,
                            num_idxs_reg=nts * P,
                            elem_size=D,
                            elem_step=P,
                        )
                        offt += nts
                        left -= nts
            sx = ""   # single tag stream (A/B split measured slower)
            out_ps = opsp.tile([D, P], F32, tag="ob" + sx)

            # chunk pipeline: B-side embed -> L1B matmul + gathered-A2
            # transposes accumulated into the same psum -> relu -> L2 ->
            # edge-major transpose -> one-hot scatter.
            nchk = teff // CT
            for pr in range(nchk // 2):
                c0 = 2 * pr
                xs = []
                for dc in range(2):
                    c = c0 + dc
                    ps_a = psp.tile([64, W], BF, tag="psa")
                    for u in range(CT):
                        g, jj, k = u // 3, u % 3, c * CT + u
                        nc.tensor.matmul(
                            out=ps_a[g * D:(g + 1) * D, jj * P:(jj + 1) * P],
                            lhsT=g_sb[:, k * D:(k + 1) * D],
                            rhs=ident[:], is_transpose=True,
                            tile_position=(0, g * D), skip_group_check=True)
                    ps_b = psp.tile([64, W], F32, tag="psb")
                    nc.tensor.matmul(out=ps_b[:], lhsT=bdW0v[:],
                                     rhs=f_sb[:, c * W:(c + 1) * W],
                                     start=True, stop=True)
                    x = sb.tile([P, W], BF, tag="x" + sx)
                    nc.scalar.activation(x[0:64, :], ps_a[:], Copy)
                    nc.vector.tensor_scalar_max(x[64:P, :], ps_b[:], 0.0)
                    xs.append(x)
                ps_h = psp.tile([P, W], F32, tag="psh" + sx)
                for q2 in range(2):
                    nc.tensor.matmul(out=ps_h[q2 * 64:(q2 + 1) * 64, :],
                                     lhsT=bdJ1ID[:], rhs=xs[q2][:],
                                     start=True, stop=True,
                                     tile_position=(0, q2 * 64),
                                     skip_group_check=True)
                h = sb.tile([P, W], BF, tag="h" + sx)
                nc.vector.tensor_scalar_max(h[:], ps_h[:], 0.0)
                ps_j = psp.tile([P, W], F32, tag="psj" + sx)
                nc.tensor.matmul(out=ps_j[:], lhsT=bdJ2[:], rhs=h[:],
                                 start=True, stop=True)
                j = sb.tile([P, W], BF, tag="j" + sx)
                nc.scalar.activation(j[:], ps_j[:], Relu, bias=bj2_4[:])
                ps_e = psp.tile([P, W], F32, tag="psh" + sx)
                for jj in range(3):
                    nc.tensor.matmul(
                        out=ps_e[:, jj * P:(jj + 1) * P],
                        lhsT=j[:, jj * P:(jj + 1) * P],
                        rhs=ident[:],
                        skip_group_check=True)
                jem = sb.tile([P, W], BF, tag="jem" + sx)
                nc.scalar.activation(jem[:], ps_e[:], Copy)
                for dc in range(2):
                    c = c0 + dc
                    cb = b * T2 + c * CT
                    S = sb.tile([P, CT * P], BF, tag="S" + sx)
                    nc.vector.tensor_tensor(
                        out=S[:],
                        in0=_nested_ap(rid2[:, cb:cb + CT], [[1, CT], [0, P]]),
                        in1=_nested_ap(iota[:], [[0, CT], [1, P]]),
                        op=IsEq)
                    for g in range(2):
                        for jj in range(3):
                            u = 3 * g + jj
                            q = 2 * dc + g
                            nc.tensor.matmul(
                                out=out_ps[:],
                                lhsT=jem[:, jj * P + q * D:jj * P + (q + 1) * D],
                                rhs=S[:, u * P:(u + 1) * P],
                                start=(c == 0 and u == 0),
                                stop=(c == nchk - 1 and u == CT - 1),
                                skip_group_check=True)

            # rep layer 2 + output MLP
            ov_sb = sb.tile([D, P], BF, tag="ov" + sx)
            nc.scalar.activation(ov_sb[:], out_ps[:], Copy)
            ps_r0 = psp.tile([64, W], F32, tag="psb")
            ps_r = ps_r0[0:D, 0:P]
            nc.tensor.matmul(out=ps_r, lhsT=WvrT[:], rhs=ov_sb[:],
                             start=True, stop=False)
            nc.tensor.matmul(out=ps_r, lhsT=WvrB[:],
                             rhs=vT[:, b * P:(b + 1) * P],
                             start=False, stop=True)
            ov2 = sb.tile([D, P], BF, tag="ov2" + sx)
            nc.scalar.activation(ov2[:], ps_r, Relu, bias=bvr[:])
            ps1_0 = psp.tile([P, W], F32, tag="psj" + sx)
            ps1 = ps1_0[0:D, 0:P]
            nc.tensor.matmul(out=ps1, lhsT=Wo1[:], rhs=ov2[:],
                             start=True, stop=True)
            h1 = sb.tile([D, P], BF, tag="h1o" + sx)
            nc.scalar.activation(h1[:], ps1, Relu, bias=bo1[:])
            ps2_0 = psp.tile([64, W], F32, tag="psb")
            ps2 = ps2_0[0:D, 0:P]
            nc.tensor.matmul(out=ps2, lhsT=Wo2[:], rhs=h1[:],
                             start=True, stop=True)
            h2 = sb.tile([D, P], BF, tag="h2o" + sx)
            nc.scalar.activation(h2[:], ps2, Relu, bias=bo2[:])
            ps3_0 = psp.tile([64, W], F32, tag="psb")
            ps3 = ps3_0[0:1, 0:P]
            nc.tensor.matmul(out=ps3, lhsT=Wo3[:], rhs=h2[:],
                             start=True, stop=True)
            nc.vector.tensor_scalar_add(out_sb[:, b * P:(b + 1) * P],
                                        ps3, bo3[:])

    nc.sync.dma_start(out=out_t[:], in_=out_sb[:])
    es.close()


# ----------------------------------------------------------------------------
# host driver
# ----------------------------------------------------------------------------

def make_in_maps(inputs, cores, bpc, CF, VF, n_nodes):
    npad = cores * bpc * P
    ns = bpc * P

    cons = np.zeros((npad, CF), np.float32)
    cons[:n_nodes] = np.asarray(inputs["cons_features"], np.float32)
    varsf = np.zeros((npad, VF), np.float32)
    varsf[:n_nodes] = np.asarray(inputs["vars_features"], np.float32)

    (f1, GR1, rid1, T1), (f2, GR2, rid2, ia2, T2, gather_plan) = preprocess(
        np.asarray(inputs["edge_indices"]), cons, varsf, cores, bpc)

    com = _host_weights(inputs, CF, VF)

    in_maps = []
    for c in range(cores):
        m = dict(com)
        m["consT"] = np.ascontiguousarray(
            cons[c * ns:(c + 1) * ns].T.astype(BF16))
        m["varsT"] = np.ascontiguousarray(
            varsf[c * ns:(c + 1) * ns].T.astype(BF16))
        m["f1"], m["rid1"] = f1[c], rid1[c]
        m["f2"], m["rid2"], m["ia2"] = f2[c], rid2[c], ia2[c]
        in_maps.append(m)
    return in_maps, T1, T2, gather_plan


def _pjrt_run(nc, in_maps, cores, iters=1):
    """Compile once via PJRT, execute `iters` times, return (out_list, times)."""
    import time
    import jax
    from jax.experimental.shard_map import shard_map
    from jax.sharding import Mesh, PartitionSpec
    from concourse.bass2jax import (
        install_neuronx_cc_hook, partition_id_tensor, _bass_exec_p)

    install_neuronx_cc_hook()
    assert nc.dbg_addr is None or not nc.dbg_callbacks
    if nc.dbg_addr is not None:
        in_maps = [
            {**m, nc.dbg_addr.name: np.zeros((1, 2), np.uint32)} for m in in_maps
        ]
    partition_name = nc.partition_id_tensor.name if nc.partition_id_tensor else None

    in_names, out_names, out_avals, zero_outs = [], [], [], []
    for alloc in nc.m.functions[0].allocations:
        if not isinstance(alloc, mybir.MemoryLocationSet):
            continue
        name = alloc.memorylocations[0].name
        if alloc.kind == "ExternalInput":
            if name != partition_name:
                in_names.append(name)
        elif alloc.kind == "ExternalOutput":
            shape = tuple(alloc.tensor_shape)
            dtype = mybir.dt.np(alloc.dtype)
            out_names.append(name)
            out_avals.append(jax.core.ShapedArray(shape, dtype))
            zero_outs.append(np.zeros(shape, dtype))
    n_params = len(in_names)
    n_outs = len(out_avals)
    all_in_names = list(in_names) + list(out_names)
    if partition_name is not None:
        all_in_names.append(partition_name)

    def _body(*args):
        operands = list(args)
        if partition_name is not None:
            operands.append(partition_id_tensor())
        outs = _bass_exec_p.bind(
            *operands,
            out_avals=tuple(out_avals),
            in_names=tuple(all_in_names),
            out_names=tuple(out_names),
            lowering_input_output_aliases=(),
            sim_require_finite=True,
            sim_require_nnan=True,
            nc=nc,
        )
        return tuple(outs)

    devices = jax.devices()[:cores]
    mesh = Mesh(np.asarray(devices), ("core",))
    in_specs = (PartitionSpec("core"),) * (n_params + n_outs)
    out_specs = (PartitionSpec("core"),) * len(out_names)
    sharded = jax.jit(
        shard_map(_body, mesh=mesh, in_specs=in_specs, out_specs=out_specs,
                  check_rep=False),
        keep_unused=True,
    )
    concat_in = [
        np.concatenate([np.asarray(in_maps[c][nm]) for c in range(cores)], axis=0)
        for nm in in_names
    ]
    from jax.sharding import NamedSharding
    shard = NamedSharding(mesh, PartitionSpec("core"))
    dev_in = [jax.device_put(a, shard) for a in concat_in]
    dev_zero = [
        jax.device_put(np.zeros((cores * z.shape[0], *z.shape[1:]), z.dtype),
                       shard)
        for z in zero_outs
    ]

    out_arrs = sharded(*dev_in, *dev_zero)
    jax.block_until_ready(out_arrs)
    times = []
    for _ in range(max(0, iters - 1)):
        t0 = time.perf_counter()
        out_arrs2 = sharded(*dev_in, *dev_zero)
        jax.block_until_ready(out_arrs2)
        times.append(time.perf_counter() - t0)
    results = [
        {nm: np.asarray(out_arrs[i]).reshape(cores, *out_avals[i].shape)[c]
         for i, nm in enumerate(out_names)}
        for c in range(cores)
    ]
    return results, times


def run(inputs, cores, bpc, n_nodes, iters=1, debug=False):
    CF = np.asarray(inputs["cons_features"]).shape[1]
    VF = np.asarray(inputs["vars_features"]).shape[1]
    in_maps, T1, T2, gather_plan = make_in_maps(inputs, cores, bpc, CF, VF, n_nodes)
    nc = build_program(cores, bpc, T1, T2, CF, VF,
                       gather_plan=gather_plan, debug=debug)
    results, times = _pjrt_run(nc, in_maps, cores, iters=iters)
    out = np.concatenate([results[c]["out"].reshape(-1) for c in range(cores)])
    out = out[:n_nodes].reshape(n_nodes, 1).astype(np.float32)
    if debug:
        return out, times, results
    return out, times


def kernel(**inputs) -> np.ndarray:
    out, _ = run(inputs, cores=8, bpc=98, n_nodes=100_000)
    return out



# revision 39
# speedup vs baseline: 1.0041x; 1.0041x over previous
"""Bipartite GNN message-passing kernel for 8 TRN2 NeuronCores (v2).

Strategy (per core, 98 blocks of 128 destination nodes each):
  - Host: sort edges by destination (cons for pass 1, vars for pass 2),
    group into 128-node blocks, pad to uniform T tiles of 128 edges.
    Pack per-edge raw features in a 2-group block-diagonal layout
    (24/20 partition rows x 384-wide chunks, ones-row for baked biases).
  - Device pass 1 (cons-sorted): one feature DMA per block; per chunk of
    768 edges: one embed matmul -> relu -> join L1 -> relu -> join L2 ->
    relu -> 6 PE transposes to edge-major -> one-hot (is_equal) segment
    scatter matmuls accumulated in PSUM per block; rep layer -> oc2.
  - AllGather oc2 row-major slices; pass 2 (vars-sorted): ONE batched
    indirect DMA gather per block (idx [128,T], flat out AP), PE
    transposes to feature-major, streamed raw B-side features, same join
    + scatter + rep + output MLP.
  - Elementwise work is spread across Act/DVE/Pool engines; biases are
    baked into matmuls via ones-rows or fused into tensor_scalar ops.
"""

import sys

sys.path.insert(0, "/opt/trn_rl_repo")

import inspect
import textwrap

import numpy as np
import ml_dtypes

import concourse.bass as bass
import concourse.tile as tile
from concourse import bacc, mybir

# dma_gather with the 256B *payload* restriction relaxed (the 256B table
# *stride* restriction is real HW ucode behavior and is kept).  Verified on
# HW: elem_size=32 bf16 payload with elem_step=128 (256B stride) gathers
# correctly when the int16 indices are replicated across the 8 x 16-partition
# groups (each GPSIMD core reads its own 16 partitions).
_src = textwrap.dedent(inspect.getsource(bass.BassGpSimd.dma_gather))
_src = _src.replace(
    "assert (\n"
    "        elem_size_bytes > 0 and elem_size_bytes % 256 == 0\n"
    "    )  # transpose restriction",
    "assert elem_size_bytes > 0")
assert "% 256 == 0" not in _src
_ns = dict(vars(bass))
exec(compile(_src, "<patched_dma_gather>", "exec"), _ns)
bass.BassGpSimd.dma_gather_small = _ns["dma_gather"]

RNG = 32768  # int16 index range per gather run (cons >> 15)

# The stock cost model says SWDGE descriptor generation costs 0.34ns/desc;
# measured on HW it is ~7.5ns/desc (the Tile scheduler uses this model to
# order instructions -- with the stock value it schedules gathers lazily).
from concourse import hw_specs as _hw_specs

_hw_specs.TRN2Spec.SWDGE_NS_PER_DESCRIPTOR = 7.5

BF16 = ml_dtypes.bfloat16
P = 128
D = 32
CT = 6          # tiles per chunk (2 groups x 3)
W = 384         # free width per chunk (3 tiles)

F32 = mybir.dt.float32
BF = mybir.dt.bfloat16
I32 = mybir.dt.int32
I16 = mybir.dt.int16
Relu = mybir.ActivationFunctionType.Relu
Copy = mybir.ActivationFunctionType.Copy
Add = mybir.AluOpType.add
Max = mybir.AluOpType.max
IsEq = mybir.AluOpType.is_equal


# ----------------------------------------------------------------------------
# host-side preprocessing
# ----------------------------------------------------------------------------

def _prep(sort_key, cores, bpc):
    """Sort edges by sort_key, assign per-block slots, T mult of CT."""
    E = sort_key.shape[0]
    nblk = cores * bpc
    order = np.argsort(sort_key, kind="stable")
    sk = sort_key[order].astype(np.int64)
    blk = sk // P
    cnt = np.bincount(blk, minlength=nblk).astype(np.int64)
    T = int(np.ceil(cnt.max() / P))
    T = (T + CT - 1) // CT * CT
    starts = np.zeros(nblk + 1, np.int64)
    np.cumsum(cnt, out=starts[1:])
    rank = np.arange(E, dtype=np.int64) - starts[blk]
    return order, sk, blk, rank, nblk, T


def _slot_cols(rank, T):
    """tile t = 6c+3g+j -> (t, g, column c*384 + j*128 + p)."""
    t = rank // P
    p = rank % P
    c = t // CT
    r = t % CT
    g = r // 3
    j = r % 3
    col = c * W + j * P + p
    return t, p, g, col


def _pack_rid(blk, sk, rank, nblk, T, cores, bpc):
    """[cores, 128, bpc*T] bf16: node offset within block, 999 for pads."""
    t, p, _, _ = _slot_cols(rank, T)
    arr = np.full((nblk, P, T), 999.0, np.float32)
    arr[blk, p, t] = (sk - blk * P).astype(np.float32)
    arr = arr.reshape(cores, bpc, P, T).transpose(0, 2, 1, 3)
    return np.ascontiguousarray(arr.reshape(cores, P, bpc * T).astype(BF16))


def _prep2_runs(ec, ev, cores, bpc):
    """Vars-sorted edges, run-partitioned by cons>>15 inside each block.

    Each block's edges are grouped into 4 runs by cons index range (so the
    per-run dma_gather int16 indices stay < 32768), runs padded to whole
    128-edge tiles with tile counts uniform across cores (SPMD program).
    """
    E = ev.shape[0]
    nblk = cores * bpc
    o = np.argsort(ev, kind="stable")
    sk = ev[o].astype(np.int64)
    blk = sk // P
    cons = ec[o].astype(np.int64)
    s = cons // RNG
    NR = 4
    order2 = np.lexsort((np.arange(E), s, blk))
    o, sk, blk, cons, s = (o[order2], sk[order2], blk[order2],
                           cons[order2], s[order2])
    len_bs = np.zeros((nblk, NR), np.int64)
    np.add.at(len_bs, (blk, s), 1)
    nt = np.ceil(len_bs.reshape(cores, bpc, NR).max(axis=0) / P)
    nt = nt.astype(np.int64)                      # [bpc, NR]
    off = np.zeros((bpc, NR), np.int64)
    off[:, 1:] = np.cumsum(nt, axis=1)[:, :-1]
    Tu = nt.sum(axis=1)
    T = int(Tu.max())
    T = (T + 2 * CT - 1) // (2 * CT) * (2 * CT)  # even chunk count
    grp = blk * NR + s
    starts = np.zeros(nblk * NR + 1, np.int64)
    np.cumsum(np.bincount(grp, minlength=nblk * NR), out=starts[1:])
    pos = np.arange(E, dtype=np.int64) - starts[grp]
    rank = off[blk % bpc, s] * P + pos
    return o, sk, blk, rank, cons, s, nt, T


def _pack_idx16(blk, rank, cons, s, nblk, T, cores, bpc):
    """[cores, bpc, 128, T*8] int16 gather indices, replicated across the
    8x16-partition groups (each GPSIMD core reads its own 16 partitions)."""
    C = T * 8
    arr = np.zeros((nblk, 16, C), np.int16)
    arr[blk, rank % 16, rank // 16] = (cons - s * RNG).astype(np.int16)
    arr = arr.reshape(cores, bpc, 16, C)
    return np.ascontiguousarray(np.tile(arr, (1, 1, 8, 1)))


def _pack_groupfeats(blk, rank, T, nblk, cores, bpc, row_blocks):
    """Pack per-edge features into [cores, bpc, GR, (T//CT)*W] bf16.

    row_blocks: list of (feat_array [E_sorted, F], row_offset_in_group).
    Group g occupies partition rows [g*gstride, (g+1)*gstride); the last
    row of each group is set to 1.0 (bias ones-row).
    """
    gstride = sum(f.shape[1] for f, _ in row_blocks) + 1
    GR = 2 * gstride
    t, p, g, col = _slot_cols(rank, T)
    nch = T // CT
    arr = np.zeros((nblk, GR, nch * W), np.float32)
    for feat, roff in row_blocks:
        F = feat.shape[1]
        for f in range(F):
            arr[blk, g * gstride + roff + f, col] = feat[:, f]
    arr[:, gstride - 1, :] = 1.0
    arr[:, 2 * gstride - 1, :] = 1.0
    out = arr.reshape(cores, bpc, GR, nch * W).astype(BF16)
    return np.ascontiguousarray(out), GR


def preprocess(edge_indices, cons_features, vars_features, cores, bpc):
    ec = np.asarray(edge_indices[0]).astype(np.int64)
    ev = np.asarray(edge_indices[1]).astype(np.int64)

    # pass 1: sorted by cons
    o1, sk1, bk1, rk1, nblk, T1 = _prep(ec, cores, bpc)
    f1, GR1 = _pack_groupfeats(
        bk1, rk1, T1, nblk, cores, bpc,
        [(cons_features[ec[o1]], 0), (vars_features[ev[o1]], 2)])
    rid1 = _pack_rid(bk1, sk1, rk1, nblk, T1, cores, bpc)

    # pass 2: sorted by vars, run-partitioned by cons range for dma_gather
    nblk = cores * bpc
    o2, sk2, bk2, rk2, cons2, s2, nt2, T2 = _prep2_runs(ec, ev, cores, bpc)
    f2, GR2 = _pack_groupfeats(
        bk2, rk2, T2, nblk, cores, bpc, [(vars_features[ev[o2]], 0)])
    rid2 = _pack_rid(bk2, sk2, rk2, nblk, T2, cores, bpc)
    ia2 = _pack_idx16(bk2, rk2, cons2, s2, nblk, T2, cores, bpc)
    gather_plan = [[int(v) for v in nt2[lb]] for lb in range(bpc)]

    return (f1, GR1, rid1, T1), (f2, GR2, rid2, ia2, T2, gather_plan)


def _bdj1id(w):
    """L1 weight for pass 2: A-half = identity (gathered A2 passes through),
    B-half = Wj1B, 2-group block-diagonal."""
    m = np.zeros((P, 64), np.float32)
    for g in range(2):
        m[g * D:(g + 1) * D, g * D:(g + 1) * D] = np.eye(D)
        m[64 + g * D:64 + (g + 1) * D, g * D:(g + 1) * D] = w["Wj1"][D:2 * D]
    return m.astype(BF16)


def _host_weights(inputs, CF, VF):
    """Block-diagonal / packed weight arrays (host-side relayout only)."""
    w = {k: np.asarray(inputs[k], np.float32) for k in
         ["Wc", "bc", "Wv", "bv", "Wj1", "bj1", "Wj2", "bj2",
          "Wcr", "bcr", "Wvr", "bvr", "Wo1", "bo1", "Wo2", "bo2",
          "Wo3", "bo3"]}
    g1 = CF + VF + 1          # 12
    bdW0 = np.zeros((2 * g1, P), np.float32)
    for g in range(2):
        a0, b0 = g * D, 64 + g * D
        bdW0[g * g1:g * g1 + CF, a0:a0 + D] = w["Wc"]
        bdW0[g * g1 + CF:g * g1 + CF + VF, b0:b0 + D] = w["Wv"]
        bdW0[g * g1 + g1 - 1, a0:a0 + D] = w["bc"]
        bdW0[g * g1 + g1 - 1, b0:b0 + D] = w["bv"]
    g2 = VF + 1               # 10
    bdW0v = np.zeros((2 * g2, 64), np.float32)
    for g in range(2):
        bdW0v[g * g2:g * g2 + VF, g * D:(g + 1) * D] = w["Wv"]
        bdW0v[g * g2 + VF, g * D:(g + 1) * D] = w["bv"]
    bdJ1 = np.zeros((P, 64), np.float32)
    for g in range(2):
        bdJ1[g * D:(g + 1) * D, g * D:(g + 1) * D] = w["Wj1"][0:D]       # A
        bdJ1[64 + g * D:64 + (g + 1) * D, g * D:(g + 1) * D] = w["Wj1"][D:2 * D]
    bdJ2 = np.zeros((P, P), np.float32)
    for g in range(4):
        bdJ2[g * D:(g + 1) * D, g * D:(g + 1) * D] = w["Wj2"]

    com = {
        "bdW0": bdW0.astype(BF16), "bdW0v": bdW0v.astype(BF16),
        "bdJ1": bdJ1.astype(BF16), "bdJ2": bdJ2.astype(BF16),
        "WcrT": w["Wcr"][0:D].astype(BF16), "WcrB": w["Wcr"][D:2 * D].astype(BF16),
        "WvrT": w["Wvr"][0:D].astype(BF16), "WvrB": w["Wvr"][D:2 * D].astype(BF16),
        "Wo1": w["Wo1"].astype(BF16), "Wo2": w["Wo2"].astype(BF16),
        "Wo3": w["Wo3"].astype(BF16),
        "WjA": w["Wj1"][0:D].astype(BF16),
        "bdJ1ID": _bdj1id(w),
        "bj1v": w["bj1"].reshape(D, 1),
        "bj1_4": np.tile(w["bj1"], 4).reshape(P, 1),
        "bj2_4": np.tile(w["bj2"], 4).reshape(P, 1),
        "bcr": w["bcr"].reshape(D, 1), "bvr": w["bvr"].reshape(D, 1),
        "bo1": w["bo1"].reshape(D, 1), "bo2": w["bo2"].reshape(D, 1),
        "bo3": w["bo3"].reshape(1, 1),
        "ident": np.eye(P, dtype=BF16),
        "iota": np.arange(P, dtype=BF16).reshape(1, P),
    }
    return com


# ----------------------------------------------------------------------------
# device program
# ----------------------------------------------------------------------------

def _bcast_row(ap, parts=P):
    return bass.AP(tensor=ap.tensor, offset=ap.offset, ap=[[0, parts]] + ap.ap[1:])


def build_program(cores, bpc, T1, T2, CF, VF, gather_plan=None, debug=False):
    npad = cores * bpc * P
    ns = bpc * P
    GR1 = 2 * (CF + VF + 1)
    GR2 = 2 * (VF + 1)
    nch1, nch2 = T1 // CT, T2 // CT
    nc = bacc.Bacc(None, num_devices=cores)

    def inp(name, shape, dt):
        return nc.dram_tensor(name, shape, dt, kind="ExternalInput")

    t = {}
    t["consT"] = inp("consT", [CF, ns], BF)
    t["varsT"] = inp("varsT", [VF, ns], BF)
    t["f1"] = inp("f1", [bpc, GR1, nch1 * W], BF)
    t["f2"] = inp("f2", [bpc, GR2, nch2 * W], BF)
    t["rid1"] = inp("rid1", [P, bpc * T1], BF)
    t["rid2"] = inp("rid2", [P, bpc * T2], BF)
    t["ia2"] = inp("ia2", [bpc, P, T2 * 8], I16)
    for nm, shp in [("bdW0", [GR1, P]), ("bdW0v", [GR2, 64]),
                    ("bdJ1", [P, 64]), ("bdJ2", [P, P]),
                    ("WcrT", [D, D]), ("WcrB", [D, D]),
                    ("WjA", [D, D]), ("bdJ1ID", [P, 64]),
                    ("WvrT", [D, D]), ("WvrB", [D, D]),
                    ("Wo1", [D, D]), ("Wo2", [D, D]), ("Wo3", [D, 1]),
                    ("ident", [P, P]), ("iota", [1, P])]:
        t[nm] = inp(nm, shp, BF)
    for nm, shp in [("bj1_4", [P, 1]), ("bj2_4", [P, 1]),
                    ("bj1v", [D, 1]),
                    ("bcr", [D, 1]), ("bvr", [D, 1]),
                    ("bo1", [D, 1]), ("bo2", [D, 1]), ("bo3", [1, 1])]:
        t[nm] = inp(nm, shp, F32)

    out_t = nc.dram_tensor("out", [1, ns], F32, kind="ExternalOutput")
    oc2_row_slice = nc.dram_tensor("oc2_row_slice", [ns, D], BF, kind="Internal")
    oc2_row_full = nc.dram_tensor(
        "oc2_row_full", [npad, D], BF, kind="Internal",
        addr_space="Shared" if cores > 4 else "Local")
    # 256B-stride gather tables (one per 32768-row cons range)
    tabs = [nc.dram_tensor(f"oc2tab{s}", [RNG, P], BF, kind="Internal")
            for s in range(4)]

    dbg = {}
    if debug:
        dbg["dbg_oc2"] = (nc.dram_tensor("dbg_oc2", [npad, D], BF,
                                         kind="ExternalOutput"), oc2_row_full)

    if gather_plan is None:
        gather_plan = [[T2, 0, 0, 0]] * bpc
    with tile.TileContext(nc) as tc:
        _emit(tc, t, out_t, oc2_row_slice, oc2_row_full, tabs,
              cores, bpc, T1, T2, CF, VF, gather_plan)
        for nm, (dst, src_t) in dbg.items():
            nc.sync.dma_start(out=dst[:], in_=src_t[:])
    nc.compile()
    return nc


def _nested_ap(base, extra_dims_after_partition):
    """Insert broadcast/repeat dims after the partition dim of an SBUF AP."""
    return bass.AP(tensor=base.tensor, offset=base.offset,
                   ap=[base.ap[0]] + extra_dims_after_partition)


def _emit(tc, t, out_t, oc2_row_slice, oc2_row_full, tabs,
          cores, bpc, T1, T2, CF, VF, gather_plan):
    nc = tc.nc
    ns = bpc * P
    GR1 = 2 * (CF + VF + 1)
    GR2 = 2 * (VF + 1)
    nch1, nch2 = T1 // CT, T2 // CT
    from contextlib import ExitStack
    es = ExitStack()
    singles = es.enter_context(tc.tile_pool(name="singles", bufs=1))

    # ---- persistent SBUF ----
    ident = singles.tile([P, P], BF)
    nc.sync.dma_start(out=ident[:], in_=t["ident"][:])
    iota = singles.tile([P, P], BF)
    nc.sync.dma_start(out=iota[:], in_=_bcast_row(t["iota"][:]))

    def load(nm, shape, dt=BF):
        w = singles.tile(shape, dt, tag=nm)
        nc.sync.dma_start(out=w[:], in_=t[nm][:])
        return w

    bdW0 = load("bdW0", [GR1, P])
    bdW0v = load("bdW0v", [GR2, 64])
    bdJ1 = load("bdJ1", [P, 64])
    bdJ2 = load("bdJ2", [P, P])
    WcrT = load("WcrT", [D, D])
    WjA = load("WjA", [D, D])
    bdJ1ID = load("bdJ1ID", [P, 64])
    bj1v = load("bj1v", [D, 1], F32)
    WcrB = load("WcrB", [D, D])
    WvrT = load("WvrT", [D, D])
    WvrB = load("WvrB", [D, D])
    Wo1 = load("Wo1", [D, D])
    Wo2 = load("Wo2", [D, D])
    Wo3 = load("Wo3", [D, 1])
    bj1_4 = load("bj1_4", [P, 1], F32)
    bj2_4 = load("bj2_4", [P, 1], F32)
    bcr = load("bcr", [D, 1], F32)
    bvr = load("bvr", [D, 1], F32)
    bo1 = load("bo1", [D, 1], F32)
    bo2 = load("bo2", [D, 1], F32)
    bo3 = load("bo3", [1, 1], F32)
    rid1 = load("rid1", [P, bpc * T1])
    rid2 = load("rid2", [P, bpc * T2])

    cT = singles.tile([D, ns], BF)       # c = relu(cons @ Wc + bc), feat-major
    vT = singles.tile([D, ns], BF)
    a2T = singles.tile([D, ns], BF)
    out_sb = singles.tile([1, ns], F32)

    # ---- stage A: node tables ----
    WcT = singles.tile([CF, D], BF)
    nc.sync.dma_start(out=WcT[:], in_=t["bdW0"][0:CF, 0:D])
    WvT = singles.tile([VF, D], BF)
    nc.sync.dma_start(out=WvT[:], in_=t["bdW0"][CF:CF + VF, 64:64 + D])
    bc_sb = singles.tile([1, D], BF)
    nc.sync.dma_start(out=bc_sb[:], in_=t["bdW0"][CF + VF:CF + VF + 1, 0:D])
    bv_sb = singles.tile([1, D], BF)
    nc.sync.dma_start(out=bv_sb[:], in_=t["bdW0"][CF + VF:CF + VF + 1, 64:64 + D])

    ones1 = singles.tile([1, 512], BF, tag="ones1")
    nc.vector.memset(ones1[:], 1.0)
    with tc.tile_pool(name="bld", bufs=1) as bld, \
         tc.tile_pool(name="bps", bufs=4, space="PSUM") as bps:
        consT = bld.tile([CF, ns], BF, tag="consT")
        nc.sync.dma_start(out=consT[:], in_=t["consT"][:])
        varsT = bld.tile([VF, ns], BF, tag="varsT")
        nc.sync.dma_start(out=varsT[:], in_=t["varsT"][:])
        for st0 in range(0, ns, 512):
            wdt = min(512, ns - st0)
            for (Wsb, src, bias, dst) in [(WcT, consT, bc_sb, cT),
                                          (WvT, varsT, bv_sb, vT)]:
                ps = bps.tile([D, 512], F32, tag="ps")
                nc.tensor.matmul(out=ps[:, :wdt], lhsT=Wsb[:],
                                 rhs=src[:, st0:st0 + wdt], start=True, stop=False)
                nc.tensor.matmul(out=ps[:, :wdt], lhsT=bias[:],
                                 rhs=ones1[:, :wdt], start=False, stop=True)
                nc.scalar.activation(dst[:, st0:st0 + wdt], ps[:, :wdt], Relu)

    # ---- shared chunk emitter (chunks processed in pairs) ----
    def do_chunks(T, x_cb, rid, rid_off, out_ps, sb, psp):
        nchk = T // CT
        assert nchk % 2 == 0
        for pr in range(nchk // 2):
            c0ch = 2 * pr
            xs = [x_cb(c0ch), x_cb(c0ch + 1)]
            # join L1 for both chunks into one [128, W] psum
            ps_h = psp.tile([P, W], F32, tag="psh")
            for q2 in range(2):
                nc.tensor.matmul(out=ps_h[q2 * 64:(q2 + 1) * 64, :],
                                 lhsT=bdJ1[:], rhs=xs[q2][:],
                                 start=True, stop=True,
                                 tile_position=(0, q2 * 64),
                                 skip_group_check=True)
            h = sb.tile([P, W], BF, tag="h")
            nc.vector.tensor_scalar(out=h[:], in0=ps_h[:], scalar1=bj1_4[:],
                                    scalar2=0.0, op0=Add, op1=Max)
            # join L2 (4 block-diagonal groups)
            ps_j = psp.tile([P, W], F32, tag="psj")
            nc.tensor.matmul(out=ps_j[:], lhsT=bdJ2[:], rhs=h[:],
                             start=True, stop=True)
            j = sb.tile([P, W], BF, tag="j")
            nc.scalar.activation(j[:], ps_j[:], Relu, bias=bj2_4[:])
            # transpose to edge-major: col-slice jj -> [128, 128] holding
            # 4 row-groups q (chunk c0ch+q//2, group q%2)
            ps_e = psp.tile([P, W], BF, tag="pse")
            for jj in range(3):
                nc.tensor.matmul(
                    out=ps_e[:, jj * P:(jj + 1) * P],
                    lhsT=j[:, jj * P:(jj + 1) * P],
                    rhs=ident[:], is_transpose=True, skip_group_check=True)
            jem = sb.tile([P, W], BF, tag="jem")
            nc.scalar.activation(jem[:], ps_e[:], Copy)
            # one-hot scatter per chunk
            for dc in range(2):
                c = c0ch + dc
                cb = rid_off + c * CT
                S = sb.tile([P, CT * P], BF, tag="S")
                nc.vector.tensor_tensor(
                    out=S[:],
                    in0=_nested_ap(rid[:, cb:cb + CT], [[1, CT], [0, P]]),
                    in1=_nested_ap(iota[:], [[0, CT], [1, P]]),
                    op=IsEq)
                for g in range(2):
                    for jj in range(3):
                        u = 3 * g + jj
                        q = 2 * dc + g
                        nc.tensor.matmul(
                            out=out_ps[:],
                            lhsT=jem[:, jj * P + q * D:jj * P + (q + 1) * D],
                            rhs=S[:, u * P:(u + 1) * P],
                            start=(c == 0 and u == 0),
                            stop=(c == nchk - 1 and u == CT - 1),
                            skip_group_check=True)

    # ---- pass 1 ----
    with tc.tile_pool(name="p1f", bufs=3) as fpool, \
         tc.tile_pool(name="p1s", bufs=3) as sb, \
         tc.tile_pool(name="p1x", bufs=2, space="PSUM") as pspx, \
         tc.tile_pool(name="p1p", bufs=1, space="PSUM") as psp, \
         tc.tile_pool(name="p1o", bufs=2, space="PSUM") as opsp:
        for b in range(bpc):
            f_sb = fpool.tile([GR1, nch1 * W], BF, tag="f1")
            nc.sync.dma_start(out=f_sb[:], in_=t["f1"][b])
            out_ps = opsp.tile([D, P], F32, tag="ob")

            def x1(c, f_sb=f_sb):
                ps_x = pspx.tile([P, W], F32, tag="psx")
                nc.tensor.matmul(out=ps_x[:], lhsT=bdW0[:],
                                 rhs=f_sb[:, c * W:(c + 1) * W],
                                 start=True, stop=True)
                x = sb.tile([P, W], BF, tag="x")
                nc.scalar.activation(x[:], ps_x[:], Relu)
                return x

            do_chunks(T1, x1, rid1, b * T1, out_ps, sb, psp)
            # rep layer 1 -> oc2T
            oc_sb = sb.tile([D, P], BF, tag="oc")
            nc.scalar.activation(oc_sb[:], out_ps[:], Copy)
            ps_r = psp.tile([D, P], F32, tag="psr")
            nc.tensor.matmul(out=ps_r[:], lhsT=WcrT[:], rhs=oc_sb[:],
                             start=True, stop=False)
            nc.tensor.matmul(out=ps_r[:], lhsT=WcrB[:],
                             rhs=cT[:, b * P:(b + 1) * P],
                             start=False, stop=True)
            oc2_blk = sb.tile([D, P], BF, tag="oc2blk")
            nc.scalar.activation(oc2_blk[:], ps_r[:], Relu, bias=bcr[:])
            # A2 = Wj1A^T oc2 + bj1 (the pass-2 join L1 A-contribution):
            # exchanging A2 instead of oc2 lets gathered rows accumulate
            # straight into the L1 psum on the consumer side.
            ps_a2 = psp.tile([D, P], F32, tag="psr")
            nc.tensor.matmul(out=ps_a2[:], lhsT=WjA[:],
                             rhs=oc2_blk[:],
                             start=True, stop=True)
            nc.vector.tensor_scalar_add(a2T[:, b * P:(b + 1) * P],
                                        ps_a2[:], bj1v[:])

    # ---- exchange ----
    with tc.tile_pool(name="xch", bufs=1) as xp:
        stg = xp.tile([P, bpc, D], BF)
        nc.sync.dma_start(out=stg[:], in_=a2T[:], transpose=True)
        nc.sync.dma_start(
            out=oc2_row_slice[:].rearrange("(j p) d -> p j d", p=P),
            in_=stg[:])
    import os as _os
    if cores > 1 and not _os.environ.get("K_SKIP_COLLECTIVE"):
        nc.gpsimd.collective_compute(
            "AllGather", mybir.AluOpType.bypass,
            replica_groups=[list(range(cores))],
            ins=[oc2_row_slice[:]], outs=[oc2_row_full[:]])
    else:
        nc.sync.dma_start(out=oc2_row_full[0:bpc * P, :], in_=oc2_row_slice[:])
    # expand the compact [npad, D] table into 4 tables with 256B row stride
    import os as _os
    npad = cores * bpc * P
    if not _os.environ.get("K_SKIP_EXPAND"):
        for s in range(4):
            rows = min(RNG, npad - s * RNG)
            if rows <= 0:
                continue
            eng = nc.sync if s % 2 == 0 else nc.scalar
            eng.dma_start(out=tabs[s][0:rows, 0:D],
                          in_=oc2_row_full[s * RNG:s * RNG + rows, :])

    # ---- pass 2 ----
    with tc.tile_pool(name="p2f", bufs=3) as fpool, \
         tc.tile_pool(name="p2i", bufs=8) as ipool, \
         tc.tile_pool(name="p2g", bufs=8) as gpool, \
         tc.tile_pool(name="p2s", bufs=3) as sb, \
         tc.tile_pool(name="p2p", bufs=1, space="PSUM") as psp, \
         tc.tile_pool(name="p2o", bufs=2, space="PSUM") as opsp:
        for b in range(bpc):
            plan = gather_plan[b]
            tg = sum(plan)
            # per-block effective tile count (global T2 is the worst case)
            teff = (tg + 2 * CT - 1) // (2 * CT) * (2 * CT)
            nchb = teff // CT
            with tc.high_priority():
                f_sb = fpool.tile([GR2, nch2 * W], BF, tag="f2")
                nc.sync.dma_start(out=f_sb[:, :nchb * W],
                                  in_=t["f2"][b, :, 0:nchb * W])
            with tc.high_priority():
                it = ipool.tile([P, T2 * 8], I16, tag="it")
                nc.scalar.dma_start(out=it[:], in_=t["ia2"][b])
                g_sb = gpool.tile([P, T2 * D], BF, tag="gth")
                if tg < teff:
                    nc.vector.memset(g_sb[:, tg * D:teff * D], 0)
                offt = 0
                for s in range(4):
                    nts_all = plan[s]
                    if nts_all == 0:
                        continue
                    ncall = (nts_all + 7) // 8  # HW limit: ~1024 idx/call
                    step = (nts_all + ncall - 1) // ncall
                    left = nts_all
                    while left > 0:
                        nts = min(left, step)
                        sl = g_sb[:, offt * D:(offt + nts) * D]
                        g3 = bass.AP(tensor=sl.tensor, offset=sl.offset,
                                     ap=[sl.ap[0], [D, nts], [1, D]])
                        nc.gpsimd.dma_gather_small(
                            out_ap=g3,
                            in_ap=tabs[s][:, 0:D],
                            idxs_ap=it[:, offt * 8:(offt + nts) * 8],
                            num_idxs=nts * P,
                            num_idxs_reg=nts * P,
                            elem_size=D,
                            elem_step=P,
                        )
                        offt += nts
                        left -= nts
            sx = ""   # single tag stream (A/B split measured slower)
            out_ps = opsp.tile([D, P], F32, tag="ob" + sx)

            # chunk pipeline: B-side embed -> L1B matmul + gathered-A2
            # transposes accumulated into the same psum -> relu -> L2 ->
            # edge-major transpose -> one-hot scatter.
            nchk = teff // CT
            for pr in range(nchk // 2):
                c0 = 2 * pr
                xs = []
                for dc in range(2):
                    c = c0 + dc
                    ps_a = psp.tile([64, W], BF, tag="psa")
                    for u in range(CT):
                        g, jj, k = u // 3, u % 3, c * CT + u
                        nc.tensor.matmul(
                            out=ps_a[g * D:(g + 1) * D, jj * P:(jj + 1) * P],
                            lhsT=g_sb[:, k * D:(k + 1) * D],
                            rhs=ident[:], is_transpose=True,
                            tile_position=(0, g * D), skip_group_check=True)
                    ps_b = psp.tile([64, W], F32, tag="psb")
                    nc.tensor.matmul(out=ps_b[:], lhsT=bdW0v[:],
                                     rhs=f_sb[:, c * W:(c + 1) * W],
                                     start=True, stop=True)
                    x = sb.tile([P, W], BF, tag="x" + sx)
                    nc.scalar.activation(x[0:64, :], ps_a[:], Copy)
                    nc.vector.tensor_scalar_max(x[64:P, :], ps_b[:], 0.0)
                    xs.append(x)
                ps_h = psp.tile([P, W], F32, tag="psh" + sx)
                for q2 in range(2):
                    nc.tensor.matmul(out=ps_h[q2 * 64:(q2 + 1) * 64, :],
                                     lhsT=bdJ1ID[:], rhs=xs[q2][:],
                                     start=True, stop=True,
                                     tile_position=(0, q2 * 64),
                                     skip_group_check=True)
                h = sb.tile([P, W], BF, tag="h" + sx)
                nc.vector.tensor_scalar_max(h[:], ps_h[:], 0.0)
                ps_j = psp.tile([P, W], F32, tag="psj" + sx)
                nc.tensor.matmul(out=ps_j[:], lhsT=bdJ2[:], rhs=h[:],
                                 start=True, stop=True)
                j = sb.tile([P, W], BF, tag="j" + sx)
                nc.scalar.activation(j[:], ps_j[:], Relu, bias=bj2_4[:])
                ps_e = psp.tile([P, W], F32, tag="psh" + sx)
                for jj in range(3):
                    nc.tensor.matmul(
                        out=ps_e[:, jj * P:(jj + 1) * P],
                        lhsT=j[:, jj * P:(jj + 1) * P],
                        rhs=ident[:],
                        skip_group_check=True)
                jem = sb.tile([P, W], BF, tag="jem" + sx)
                nc.scalar.activation(jem[:], ps_e[:], Copy)
                for dc in range(2):
                    c = c0 + dc
                    cb = b * T2 + c * CT
                    S = sb.tile([P, CT * P], BF, tag="S" + sx)
                    nc.vector.tensor_tensor(
                        out=S[:],
                        in0=_nested_ap(rid2[:, cb:cb + CT], [[1, CT], [0, P]]),
                        in1=_nested_ap(iota[:], [[0, CT], [1, P]]),
                        op=IsEq)
                    for g in range(2):
                        for jj in range(3):
                            u = 3 * g + jj
                            q = 2 * dc + g
                            nc.tensor.matmul(
                                out=out_ps[:],
                                lhsT=jem[:, jj * P + q * D:jj * P + (q + 1) * D],
                                rhs=S[:, u * P:(u + 1) * P],
                                start=(c == 0 and u == 0),
                                stop=(c == nchk - 1 and u == CT - 1),
                                skip_group_check=True)

            # rep layer 2 + output MLP
            ov_sb = sb.tile([D, P], BF, tag="ov" + sx)
            nc.scalar.activation(ov_sb[:], out_ps[:], Copy)
            ps_r0 = psp.tile([64, W], F32, tag="psb")
            ps_r = ps_r0[0:D, 0:P]
            nc.tensor.matmul(out=ps_r, lhsT=WvrT[:], rhs=ov_sb[:],
                             start=True, stop=False)
            nc.tensor.matmul(out=ps_r, lhsT=WvrB[:],
                             rhs=vT[:, b * P:(b + 1) * P],
                             start=False, stop=True)
            ov2 = sb.tile([D, P], BF, tag="ov2" + sx)
            nc.scalar.activation(ov2[:], ps_r, Relu, bias=bvr[:])
            ps1_0 = psp.tile([P, W], F32, tag="psj" + sx)
            ps1 = ps1_0[0:D, 0:P]
            nc.tensor.matmul(out=ps1, lhsT=Wo1[:], rhs=ov2[:],
                             start=True, stop=True)
            h1 = sb.tile([D, P], BF, tag="h1o" + sx)
            nc.scalar.activation(h1[:], ps1, Relu, bias=bo1[:])
            ps2_0 = psp.tile([64, W], F32, tag="psb")
            ps2 = ps2_0[0:D, 0:P]
            nc.tensor.matmul(out=ps2, lhsT=Wo2[:], rhs=h1[:],
                             start=True, stop=True)
            h2 = sb.tile([D, P], BF, tag="h2o" + sx)
            nc.scalar.activation(h2[:], ps2, Relu, bias=bo2[:])
            ps3_0 = psp.tile([64, W], F32, tag="psb")
            ps3 = ps3_0[0:1, 0:P]
            nc.tensor.matmul(out=ps3, lhsT=Wo3[:], rhs=h2[:],
                             start=True, stop=True)
            nc.vector.tensor_scalar_add(out_sb[:, b * P:(b + 1) * P],
                                        ps3, bo3[:])

    nc.sync.dma_start(out=out_t[:], in_=out_sb[:])
    es.close()


# ----------------------------------------------------------------------------
# host driver
# ----------------------------------------------------------------------------

def make_in_maps(inputs, cores, bpc, CF, VF, n_nodes):
    npad = cores * bpc * P
    ns = bpc * P

    cons = np.zeros((npad, CF), np.float32)
    cons[:n_nodes] = np.asarray(inputs["cons_features"], np.float32)
    varsf = np.zeros((npad, VF), np.float32)
    varsf[:n_nodes] = np.asarray(inputs["vars_features"], np.float32)

    (f1, GR1, rid1, T1), (f2, GR2, rid2, ia2, T2, gather_plan) = preprocess(
        np.asarray(inputs["edge_indices"]), cons, varsf, cores, bpc)

    com = _host_weights(inputs, CF, VF)

    in_maps = []
    for c in range(cores):
        m = dict(com)
        m["consT"] = np.ascontiguousarray(
            cons[c * ns:(c + 1) * ns].T.astype(BF16))
        m["varsT"] = np.ascontiguousarray(
            varsf[c * ns:(c + 1) * ns].T.astype(BF16))
        m["f1"], m["rid1"] = f1[c], rid1[c]
        m["f2"], m["rid2"], m["ia2"] = f2[c], rid2[c], ia2[c]
        in_maps.append(m)
    return in_maps, T1, T2, gather_plan


def _pjrt_run(nc, in_maps, cores, iters=1):
    """Compile once via PJRT, execute `iters` times, return (out_list, times)."""
    import time
    import jax
    from jax.experimental.shard_map import shard_map
    from jax.sharding import Mesh, PartitionSpec
    from concourse.bass2jax import (
        install_neuronx_cc_hook, partition_id_tensor, _bass_exec_p)

    install_neuronx_cc_hook()
    assert nc.dbg_addr is None or not nc.dbg_callbacks
    if nc.dbg_addr is not None:
        in_maps = [
            {**m, nc.dbg_addr.name: np.zeros((1, 2), np.uint32)} for m in in_maps
        ]
    partition_name = nc.partition_id_tensor.name if nc.partition_id_tensor else None

    in_names, out_names, out_avals, zero_outs = [], [], [], []
    for alloc in nc.m.functions[0].allocations:
        if not isinstance(alloc, mybir.MemoryLocationSet):
            continue
        name = alloc.memorylocations[0].name
        if alloc.kind == "ExternalInput":
            if name != partition_name:
                in_names.append(name)
        elif alloc.kind == "ExternalOutput":
            shape = tuple(alloc.tensor_shape)
            dtype = mybir.dt.np(alloc.dtype)
            out_names.append(name)
            out_avals.append(jax.core.ShapedArray(shape, dtype))
            zero_outs.append(np.zeros(shape, dtype))
    n_params = len(in_names)
    n_outs = len(out_avals)
    all_in_names = list(in_names) + list(out_names)
    if partition_name is not None:
        all_in_names.append(partition_name)

    def _body(*args):
        operands = list(args)
        if partition_name is not None:
            operands.append(partition_id_tensor())
        outs = _bass_exec_p.bind(
            *operands,
            out_avals=tuple(out_avals),
            in_names=tuple(all_in_names),
            out_names=tuple(out_names),
            lowering_input_output_aliases=(),
            sim_require_finite=True,
            sim_require_nnan=True,
            nc=nc,
        )
        return tuple(outs)

    devices = jax.devices()[:cores]
    mesh = Mesh(np.asarray(devices), ("core",))
    in_specs = (PartitionSpec("core"),) * (n_params + n_outs)
    out_specs = (PartitionSpec("core"),) * len(out_names)
    sharded = jax.jit(
        shard_map(_body, mesh=mesh, in_specs=in_specs, out_specs=out_specs,
                  check_rep=False),
        keep_unused=True,
    )
    concat_in = [
        np.concatenate([np.asarray(in_maps[c][nm]) for c in range(cores)], axis=0)
        for nm in in_names
    ]
    from jax.sharding import NamedSharding
    shard = NamedSharding(mesh, PartitionSpec("core"))
    dev_in = [jax.device_put(a, shard) for a in concat_in]
    dev_zero = [
        jax.device_put(np.zeros((cores * z.shape[0], *z.shape[1:]), z.dtype),
                       shard)
        for z in zero_outs
    ]

    out_arrs = sharded(*dev_in, *dev_zero)
    jax.block_until_ready(out_arrs)
    times = []
    for _ in range(max(0, iters - 1)):
        t0 = time.perf_counter()
        out_arrs2 = sharded(*dev_in, *dev_zero)
        jax.block_until_ready(out_arrs2)
        times.append(time.perf_counter() - t0)
    results = [
        {nm: np.asarray(out_arrs[i]).reshape(cores, *out_avals[i].shape)[c]
         for i, nm in enumerate(out_names)}
        for c in range(cores)
    ]
    return results, times


def run(inputs, cores, bpc, n_nodes, iters=1, debug=False):
    CF = np.asarray(inputs["cons_features"]).shape[1]
    VF = np.asarray(inputs["vars_features"]).shape[1]
    in_maps, T1, T2, gather_plan = make_in_maps(inputs, cores, bpc, CF, VF, n_nodes)
    nc = build_program(cores, bpc, T1, T2, CF, VF,
                       gather_plan=gather_plan, debug=debug)
    results, times = _pjrt_run(nc, in_maps, cores, iters=iters)
    out = np.concatenate([results[c]["out"].reshape(-1) for c in range(cores)])
    out = out[:n_nodes].reshape(n_nodes, 1).astype(np.float32)
    if debug:
        return out, times, results
    return out, times


def kernel(**inputs) -> np.ndarray:
    out, _ = run(inputs, cores=8, bpc=98, n_nodes=100_000)
    return out

